# revision 1
# baseline (speedup 1.0000x reference)
"""Trainium2 Bass kernel for the Mamba-style block nn_Block_29721173688983.

Strategy: tensor-parallel over d_inner (2048 channels / 8 cores = 256 each).
Each core: RMSNorm (redundant), its w_in column slice, depthwise conv, silu,
partial x_proj contraction -> on-device AllReduce (the only collective),
delta via softplus Taylor poly, selective scan via DVE tensor_tensor_scan
(n-major lane layout: 16 state dims x 2 channel blocks of 128, L chunked for
pipelining), y = sum_n via identity-matmul PSUM accumulation, out_proj
partial matmul.  Host sums the 8 partial outputs and adds the residual.

kernel(**inputs) takes the FULL unsharded inputs from setup_inputs() and
returns the FULL (1, 2048, 1024) output.
"""

import sys

sys.path.insert(0, "/opt/trn_rl_repo")

from contextlib import ExitStack

import numpy as np

import concourse.bacc as bacc
import concourse.bass as bass
import concourse.tile as tile
from concourse import mybir
from concourse.bass_utils import run_bass_kernel_spmd

F32 = mybir.dt.float32
F32R = mybir.dt.float32r
AF = mybir.ActivationFunctionType
OP = mybir.AluOpType

CORES = 8
D = 1024
DI = 2048
CL = DI // CORES          # 256 channels per core
NB = CL // 128            # 2 channel blocks
NST = 16                  # d_state
DTR = 64                  # dt_rank
KCONV = 4
EPS = 1e-5
LN2 = 0.6931471805599453


class _StopBuild(Exception):
    pass


def build(L=2048, collective=True, stop_after="D", reps=1):
    """Emit the SPMD single-core program (same program on all 8 cores)."""
    LTS = min(512, L)          # psum free-dim tile
    NLT = L // LTS
    KB = D // 128              # 8 k-blocks for the w_in matmul
    LC = min(1024, L)          # phase-C scan chunk
    NCH = L // LC

    nc = bacc.Bacc("TRN2", target_bir_lowering=False, debug=False,
                   num_devices=CORES if collective else 1)

    def din(name, shape, dt=F32):
        return nc.dram_tensor(name, shape, dt, kind="ExternalInput").ap()

    xT_d = din("xT", [D, L], F32R)
    w_in_d = din("w_in_pack", [128, KB * 512], F32R)        # [p, (kb, m*128)]
    cw_d = din("cw_pack", [128, NB * KCONV])
    cbias_d = din("cbias_pack", [128, NB])
    A_d = din("A_pack", [128, NB * NST])
    D_d = din("D_pack", [128, NB])
    wxp_d = din("wxp_pack", [128, NB * (DTR + 2 * NST)], F32R)
    wdt_d = din("wdt_loc", [DTR, CL])
    bdt_d = din("bdt_pack", [128, NB])
    wout_d = din("wout_pack", [128, NB * D], F32R)
    ident_d = din("ident", [128, 128], F32R)
    ones_d = din("ones_in", [128, 1], F32R)
    cwdiag_d = din("cwdiag_pack", [128, NB * KCONV * 128], F32R)

    pout_d = nc.dram_tensor("part_out", [D, L], F32, kind="ExternalOutput").ap()

    NPROJ = DTR + 2 * NST      # 96

    with tile.TileContext(nc) as tc:
      with ExitStack() as ctx:
        try:
            # ---- persistent pools ----
            cpool = ctx.enter_context(tc.tile_pool(name="consts", bufs=1))
            drpool = ctx.enter_context(
                tc.tile_pool(name="dram", bufs=1, space="DRAM"))

            cw_t = cpool.tile([128, NB * KCONV], F32)
            cbias_t = cpool.tile([128, NB], F32)
            A_t = cpool.tile([128, NB * NST], F32)
            Aneg_t = cpool.tile([128, NB * NST], F32)
            D_t = cpool.tile([128, NB], F32)
            wxp_t = cpool.tile([128, NB * NPROJ], F32R)
            wdt_t = cpool.tile([DTR, CL], F32)
            bdt_t = cpool.tile([128, NB], F32)
            wout_t = cpool.tile([128, NB * D], F32R)
            ident_t = cpool.tile([128, 128], F32R)
            ones_t = cpool.tile([128, 1], F32R)
            cwdiag_t = cpool.tile([128, NB * KCONV * 128], F32R)

            nc.sync.dma_start(cw_t[:], cw_d[:])
            nc.sync.dma_start(cbias_t[:], cbias_d[:])
            nc.sync.dma_start(A_t[:], A_d[:])
            nc.sync.dma_start(D_t[:], D_d[:])
            nc.sync.dma_start(wxp_t[:], wxp_d[:])
            nc.sync.dma_start(wdt_t[:], wdt_d[:])
            nc.sync.dma_start(bdt_t[:], bdt_d[:])
            nc.sync.dma_start(wout_t[:], wout_d[:])
            nc.sync.dma_start(ident_t[:], ident_d[:])
            nc.sync.dma_start(ones_t[:], ones_d[:])
            nc.sync.dma_start(cwdiag_t[:], cwdiag_d[:])

            # A = -exp(A_log)
            nc.scalar.activation(Aneg_t[:], A_t[:], AF.Exp)
            nc.scalar.mul(Aneg_t[:], Aneg_t[:], -1.0)

            for rep in range(reps):
              with ExitStack() as rctx:
                # per-rep persistent activations (live through phase C)
                ppool = rctx.enter_context(
                    tc.tile_pool(name=f"persist{rep}", bufs=1))
                ar_in = drpool.tile([NPROJ, L], F32, name=f"ar_in{rep}")
                ar_out = drpool.tile([NPROJ, L], F32, addr_space="Shared",
                                     name=f"ar_out{rep}")
                rinv_dram = drpool.tile([1, L], F32, name=f"rinv_dram{rep}")
                res_silu = [ppool.tile([128, L], F32,
                                       name=f"res_silu{i}_{rep}")
                            for i in range(NB)]

                # xs_pad + rinv_bc live only through A/B: right-side pool
                padpool_ctx = ExitStack()
                padpool = padpool_ctx.enter_context(
                    tc.tile_pool(name=f"pad{rep}", bufs=1, side="right"))
                xs_pad = [padpool.tile([128, L + KCONV - 1], F32R,
                                       name=f"xs_pad{i}_{rep}")
                          for i in range(NB)]
                rinv_bc = padpool.tile([128, L], F32, name=f"rinv_bc{rep}")
                for cb in range(NB):
                    nc.vector.memset(xs_pad[cb][:, 0:KCONV - 1].bitcast(F32), 0.0)

                # ================= Phase A =================
                # Load xT; sum x^2 via ones-matmul; rinv via Newton rsqrt; main
                # matmul on UNSCALED xT (starts as soon as each block lands),
                # rinv applied on PSUM evacuation.
                with ExitStack() as actx:
                    wpool = actx.enter_context(tc.tile_pool(name="w_in", bufs=1))
                    xtpool = actx.enter_context(tc.tile_pool(name="xt", bufs=KB))
                    sqpool = actx.enter_context(tc.tile_pool(name="sq", bufs=2))
                    sspool = actx.enter_context(
                        tc.tile_pool(name="ps_ss", bufs=NLT, space="PSUM"))
                    mmpool = actx.enter_context(
                        tc.tile_pool(name="ps_mm", bufs=4, space="PSUM"))
                    rowpool = actx.enter_context(tc.tile_pool(name="rows", bufs=1))
                    evpool = actx.enter_context(tc.tile_pool(name="ev", bufs=3))

                    w_in_t = wpool.tile([128, KB * 512], F32R)

                    ss_ps = [sspool.tile([1, LTS], F32, tag="ss", name=f"ss{i}_{rep}")
                             for i in range(NLT)]
                    xt_ts = []
                    half = L // 2
                    for kb in range(KB):
                        xt = xtpool.tile([128, L], F32R, tag="xt")
                        nc.sync.dma_start(xt[:, 0:half],
                                          xT_d[bass.ts(kb, 128), 0:half])
                        nc.sync.dma_start(xt[:, half:L],
                                          xT_d[bass.ts(kb, 128), half:L])
                        xt_ts.append(xt)
                        sq = sqpool.tile([128, L], F32R, tag="sq")
                        nc.scalar.square(sq[:], xt[:])
                        for lt in range(NLT):
                            nc.tensor.matmul(
                                ss_ps[lt][:], ones_t[:],
                                sq[:, bass.ts(lt, LTS)],
                                start=(kb == 0), stop=(kb == KB - 1))

                    nc.sync.dma_start(w_in_t[:, 0:KB * 256], w_in_d[:, 0:KB * 256])
                    nc.sync.dma_start(w_in_t[:, KB * 256:], w_in_d[:, KB * 256:])

                    # rinv = rsqrt(ss/D + EPS): 2 Newton iters on [128, L/128]
                    rinv_row = rowpool.tile([1, L], F32)
                    for lt in range(NLT):
                        nc.scalar.activation(
                            rinv_row[:, bass.ts(lt, LTS)], ss_ps[lt][:],
                            AF.Copy, bias=EPS, scale=1.0 / D)
                    nc.sync.dma_start(rinv_dram[:], rinv_row[:])
                    LF = L // 128
                    m_t = rowpool.tile([128, LF], F32)
                    nc.sync.dma_start(
                        m_t[:], rinv_dram[:].rearrange("a (p f) -> (a p) f", p=128))
                    y_t = rowpool.tile([128, LF], F32)
                    t_t = rowpool.tile([128, LF], F32)
                    nc.vector.tensor_scalar(y_t[:], m_t[:], -0.5, 1.5,
                                            op0=OP.mult, op1=OP.add)
                    for _ in range(2):
                        nc.vector.tensor_mul(t_t[:], y_t[:], y_t[:])
                        nc.vector.tensor_mul(t_t[:], t_t[:], m_t[:])
                        nc.vector.tensor_scalar(t_t[:], t_t[:], -0.5, 1.5,
                                                op0=OP.mult, op1=OP.add)
                        nc.vector.tensor_mul(y_t[:], y_t[:], t_t[:])
                    nc.sync.dma_start(
                        rinv_dram[:].rearrange("a (p f) -> (a p) f", p=128), y_t[:])
                    nc.sync.dma_start(rinv_bc[:],
                                      rinv_dram[:].partition_broadcast(128))

                    # main matmul on raw xT; scale by rinv on evacuation.
                    for m in range(2 * NB):
                        for lt in range(NLT):
                            mm_ps = mmpool.tile([128, LTS], F32, tag="mm")
                            for kb in range(KB):
                                nc.tensor.matmul(
                                    mm_ps[:],
                                    w_in_t[:, kb * 512 + m * 128:
                                           kb * 512 + (m + 1) * 128],
                                    xt_ts[kb][:, bass.ts(lt, LTS)],
                                    start=(kb == 0), stop=(kb == KB - 1))
                            if m < NB:
                                nc.vector.tensor_mul(
                                    xs_pad[m][:, KCONV - 1 + lt * LTS:
                                              KCONV - 1 + (lt + 1) * LTS],
                                    mm_ps[:], rinv_bc[:, bass.ts(lt, LTS)])
                            else:
                                ev = evpool.tile([128, LTS], F32, tag="ev")
                                nc.vector.tensor_mul(
                                    ev[:], mm_ps[:], rinv_bc[:, bass.ts(lt, LTS)])
                                nc.scalar.activation(
                                    res_silu[m - NB][:, bass.ts(lt, LTS)], ev[:],
                                    AF.Silu)

                if stop_after == "A":
                    padpool_ctx.close()
                    raise _StopBuild()

                # ================= Phase B =================
                apool = rctx.enter_context(tc.tile_pool(name=f"acts{rep}", bufs=1))
                xs_silu = [apool.tile([128, L], F32R, name=f"xs_silu{i}_{rep}")
                           for i in range(NB)]
                delta = [apool.tile([128, L], F32, name=f"delta{i}_{rep}")
                         for i in range(NB)]
                u_t = [apool.tile([128, L], F32, name=f"u{i}_{rep}") for i in range(NB)]
                with ExitStack() as bctx:
                    convpool = bctx.enter_context(tc.tile_pool(name="conv", bufs=2))
                    prpool = bctx.enter_context(tc.tile_pool(name="proj", bufs=1))
                    pspool = bctx.enter_context(
                        tc.tile_pool(name="ps_b", bufs=2, space="PSUM"))

                    # depthwise causal conv on PE: 4 PSUM-accumulated
                    # diagonal matmuls per (cb, lt); Silu on evacuation.
                    for cb in range(NB):
                        for lt in range(NLT):
                            c_ps = pspool.tile([128, LTS], F32, tag="cps")
                            for j in range(KCONV):
                                nc.tensor.matmul(
                                    c_ps[:],
                                    cwdiag_t[:, (cb * KCONV + j) * 128:
                                             (cb * KCONV + j + 1) * 128],
                                    xs_pad[cb][:, j + lt * LTS:
                                               j + lt * LTS + LTS],
                                    start=(j == 0), stop=(j == KCONV - 1))
                            nc.scalar.activation(
                                xs_silu[cb][:, bass.ts(lt, LTS)], c_ps[:],
                                AF.Silu, bias=cbias_t[:, cb:cb + 1])

                    padpool_ctx.close()

                    proj_sb = prpool.tile([NPROJ, L], F32)
                    for lt in range(NLT):
                        pr_ps = pspool.tile([NPROJ, LTS], F32, tag="prps")
                        for cb in range(NB):
                            nc.tensor.matmul(
                                pr_ps[:],
                                wxp_t[:, cb * NPROJ:(cb + 1) * NPROJ],
                                xs_silu[cb][:, bass.ts(lt, LTS)],
                                start=(cb == 0), stop=(cb == NB - 1))
                        nc.scalar.copy(proj_sb[:, bass.ts(lt, LTS)], pr_ps[:])

                    nc.sync.dma_start(ar_in[:], proj_sb[:])
                    if collective:
                        nc.gpsimd.collective_compute(
                            "AllReduce", OP.add,
                            replica_groups=[list(range(CORES))],
                            ins=[ar_in.opt()], outs=[ar_out.opt()])
                    else:
                        nc.sync.dma_start(ar_out[:], ar_in[:])

                    # delta = softplus(z), z = delta_lr @ w_dt + b_dt.  No
                    # softplus table; |z| < ~0.5, Taylor:
                    # ln2 + z/2 + z^2/8 - z^4/192.
                    pd_sb = prpool.tile([DTR, L], F32)
                    nc.sync.dma_start(pd_sb[:], ar_out[0:DTR, :])
                    for cb in range(NB):
                        z_t = prpool.tile([128, L], F32, name=f"z{cb}_{rep}", tag="spz")
                        z2_t = prpool.tile([128, L], F32, name=f"z2{cb}_{rep}",
                                           tag="spz2")
                        q_t = prpool.tile([128, L], F32, name=f"q{cb}_{rep}", tag="spq")
                        for lt in range(NLT):
                            ls = bass.ts(lt, LTS)
                            d_ps = pspool.tile([128, LTS], F32, tag="dps")
                            nc.tensor.matmul(
                                d_ps[:], wdt_t[:, bass.ts(cb, 128)],
                                pd_sb[:, ls],
                                start=True, stop=True)
                            nc.scalar.activation(
                                z_t[:, ls], d_ps[:],
                                AF.Identity, bias=bdt_t[:, cb:cb + 1])
                            nc.scalar.activation(
                                z2_t[:, ls], d_ps[:],
                                AF.Square, bias=bdt_t[:, cb:cb + 1])
                            # softplus Taylor per-lt so it pipelines with the
                            # delta matmuls instead of serializing after them
                            nc.vector.tensor_scalar(
                                q_t[:, ls], z2_t[:, ls], -1.0 / 192, 0.125,
                                op0=OP.mult, op1=OP.add)
                            nc.vector.tensor_mul(q_t[:, ls], q_t[:, ls],
                                                 z2_t[:, ls])
                            nc.vector.tensor_scalar(z_t[:, ls], z_t[:, ls],
                                                    0.5, LN2,
                                                    op0=OP.mult, op1=OP.add)
                            nc.vector.tensor_add(delta[cb][:, ls], q_t[:, ls],
                                                 z_t[:, ls])
                            nc.gpsimd.tensor_tensor(
                                u_t[cb][:, ls], delta[cb][:, ls],
                                xs_silu[cb][:, ls], op=OP.mult)

                if stop_after == "B":
                    raise _StopBuild()

                # ================= Phase C: scan =================
                # Lanes: [128 channels] x (n in 16, cb in 2, chunk in NCH).
                ypool_ctx = ExitStack()
                ypool = ypool_ctx.enter_context(
                    tc.tile_pool(name="ps_y", bufs=NB * NLT, space="PSUM"))
                y_ps = [[ypool.tile([128, LTS], F32, tag="yps",
                                    name=f"yps{cb}_{lt}_{rep}")
                         for lt in range(NLT)] for cb in range(NB)]

                with ExitStack() as cctx:
                    bcpool = cctx.enter_context(tc.tile_pool(name="bc", bufs=2))
                    scpool = cctx.enter_context(tc.tile_pool(name="scw", bufs=4))
                    zpool = cctx.enter_context(tc.tile_pool(name="zw", bufs=4))

                    for n in range(NST):
                        Bb = bcpool.tile([128, L], F32, tag="Bb")
                        Cb = bcpool.tile([128, L], F32, tag="Cb")
                        for lt in range(NLT):
                            nc.sync.dma_start(
                                Bb[:, bass.ts(lt, LTS)],
                                ar_out[DTR + n:DTR + n + 1, bass.ts(lt, LTS)]
                                .partition_broadcast(128))
                            nc.sync.dma_start(
                                Cb[:, bass.ts(lt, LTS)],
                                ar_out[DTR + NST + n:DTR + NST + n + 1,
                                       bass.ts(lt, LTS)]
                                .partition_broadcast(128))
                        for cb in range(NB):
                            idx = n * NB + cb
                            ys_prev = None
                            for ch in range(NCH):
                                cs = slice(ch * LC, (ch + 1) * LC)
                                da = scpool.tile([128, LC], F32, tag="da")
                                nc.scalar.activation(
                                    da[:], delta[cb][:, cs], AF.Exp,
                                    scale=Aneg_t[:, cb * NST + n:cb * NST + n + 1])
                                dbx = scpool.tile([128, LC], F32, tag="dbxz")
                                if idx % 2 == 0:
                                    nc.gpsimd.tensor_tensor(
                                        dbx[:], u_t[cb][:, cs], Bb[:, cs],
                                        op=OP.mult)
                                else:
                                    nc.vector.tensor_mul(
                                        dbx[:], u_t[cb][:, cs], Bb[:, cs])
                                ys = scpool.tile([128, LC], F32, tag="ys")
                                nc.vector.tensor_tensor_scan(
                                    ys[:], da[:], dbx[:],
                                    0.0 if ch == 0 else ys_prev[:, LC - 1:LC],
                                    op0=OP.mult, op1=OP.add)
                                ys_prev = ys
                                z = zpool.tile([128, LC], F32R, tag="z")
                                if idx % 2 == 0:
                                    nc.vector.tensor_mul(z[:], ys[:], Cb[:, cs])
                                else:
                                    nc.gpsimd.tensor_tensor(
                                        z[:], ys[:], Cb[:, cs], op=OP.mult)
                                for q in range(LC // LTS):
                                    lt = (ch * LC) // LTS + q
                                    nc.tensor.matmul(
                                        y_ps[cb][lt][:], ident_t[:],
                                        z[:, bass.ts(q, LTS)],
                                        start=(n == 0), stop=(n == NST - 1))

                # ====== consume y psum into fin ======
                fpool = rctx.enter_context(tc.tile_pool(name=f"fin{rep}", bufs=1))
                fin = [fpool.tile([128, L], F32R, name=f"fin{i}_{rep}")
                       for i in range(NB)]
                for cb in range(NB):
                    for lt in range(NLT):
                        nc.vector.scalar_tensor_tensor(
                            fin[cb][:, bass.ts(lt, LTS)],
                            xs_silu[cb][:, bass.ts(lt, LTS)],
                            D_t[:, cb:cb + 1], y_ps[cb][lt][:],
                            op0=OP.mult, op1=OP.add)
                    nc.vector.tensor_mul(fin[cb][:], fin[cb][:], res_silu[cb][:])
                ypool_ctx.close()

                if stop_after == "C":
                    raise _StopBuild()

                # ============= Phase D: out projection =============
                with ExitStack() as dctx:
                    opool = dctx.enter_context(tc.tile_pool(name="po", bufs=3))
                    opspool = dctx.enter_context(
                        tc.tile_pool(name="ps_o", bufs=3, space="PSUM"))

                    for m in range(D // 128):
                        po_sb = opool.tile([128, L], F32, tag="po")
                        for lt in range(NLT):
                            o_ps = opspool.tile([128, LTS], F32, tag="ops")
                            for cb in range(NB):
                                nc.tensor.matmul(
                                    o_ps[:],
                                    wout_t[:, cb * D + m * 128:
                                           cb * D + (m + 1) * 128],
                                    fin[cb][:, bass.ts(lt, LTS)],
                                    start=(cb == 0), stop=(cb == NB - 1))
                            nc.scalar.copy(po_sb[:, bass.ts(lt, LTS)], o_ps[:])
                        nc.sync.dma_start(pout_d[bass.ts(m, 128), :], po_sb[:])
        except _StopBuild:
            pass

    nc.compile()
    return nc


def host_prep(inputs, L=2048):
    """Slice/replicate the full inputs into 8 per-core input maps."""
    x = np.asarray(inputs["x"], np.float32)
    norm_scale = np.asarray(inputs["norm_scale"], np.float32)
    w_in = np.asarray(inputs["w_in"], np.float32)
    conv_w = np.asarray(inputs["conv_w"], np.float32)
    conv_b = np.asarray(inputs["conv_b"], np.float32)
    A_log = np.asarray(inputs["A_log"], np.float32)
    D_in = np.asarray(inputs["D"], np.float32)
    w_xproj = np.asarray(inputs["w_xproj"], np.float32)
    w_dt = np.asarray(inputs["w_dt"], np.float32)
    b_dt = np.asarray(inputs["b_dt"], np.float32)
    w_out = np.asarray(inputs["w_out"], np.float32)

    x2 = x[0, :L, :]                              # (L, D)
    xT = np.ascontiguousarray(x2.T)               # (D, L)
    w_in_s = w_in * norm_scale[:, None]
    ident = np.eye(128, dtype=np.float32)
    KB = D // 128

    def pack_nb(v):                                # (CL,) -> [128, NB]
        return np.ascontiguousarray(v.reshape(NB, 128).T)

    in_maps = []
    for k in range(CORES):
        sl = slice(k * CL, (k + 1) * CL)
        wi = np.concatenate(
            [w_in_s[:, k * CL:(k + 1) * CL],
             w_in_s[:, DI + k * CL:DI + (k + 1) * CL]], axis=1)  # (D, 512)
        w_in_pack = np.ascontiguousarray(
            wi.reshape(KB, 128, 512).transpose(1, 0, 2).reshape(128, KB * 512))
        cw = conv_w[:, 0, sl]                     # (4, CL)
        cw_pack = np.ascontiguousarray(
            cw.reshape(KCONV, NB, 128).transpose(2, 1, 0)
            .reshape(128, NB * KCONV))
        A_pack = np.ascontiguousarray(
            A_log[sl].reshape(NB, 128, NST).transpose(1, 0, 2)
            .reshape(128, NB * NST))
        wxp_pack = np.ascontiguousarray(
            w_xproj[sl].reshape(NB, 128, DTR + 2 * NST)
            .transpose(1, 0, 2).reshape(128, NB * (DTR + 2 * NST)))
        wout_pack = np.ascontiguousarray(
            w_out[sl].reshape(NB, 128, D).transpose(1, 0, 2)
            .reshape(128, NB * D))
        in_maps.append({
            "xT": xT,
            "w_in_pack": w_in_pack,
            "cw_pack": cw_pack,
            "cbias_pack": pack_nb(conv_b[sl]),
            "A_pack": A_pack,
            "D_pack": pack_nb(D_in[sl]),
            "wxp_pack": wxp_pack,
            "wdt_loc": np.ascontiguousarray(w_dt[:, sl]),
            "bdt_pack": pack_nb(b_dt[sl]),
            "wout_pack": wout_pack,
            "ident": ident,
            "ones_in": np.ones((128, 1), np.float32),
            "cwdiag_pack": np.concatenate(
                [np.diag(cw[j, cb * 128:(cb + 1) * 128]).astype(np.float32)
                 for cb in range(NB) for j in range(KCONV)], axis=1),
        })
    return in_maps


def combine(inputs, results, L=2048):
    """Host unshard: sum the 8 partial outputs, add residual."""
    x = np.asarray(inputs["x"], np.float32)
    acc = np.zeros((D, L), np.float32)
    for r in results:
        acc += r["part_out"]
    out = x[0, :L, :] + acc.T
    return out[None].astype(np.float32)


_CACHE = {}


def kernel(**inputs):
    if "nc" not in _CACHE:
        _CACHE["nc"] = build()
    nc = _CACHE["nc"]
    in_maps = host_prep(inputs)
    res = run_bass_kernel_spmd(nc, in_maps, list(range(CORES)))
    return combine(inputs, res.results)


if __name__ == "__main__":
    import reference

    inputs = reference.setup_inputs()
    inputs = {k: np.asarray(v) for k, v in inputs.items()}
    expected = np.asarray(reference.reference(**inputs))
    actual = kernel(**inputs)
    err = np.abs(actual - expected).max() / np.abs(expected).max()
    print("Relative error:", err)



# revision 17
# speedup vs baseline: 1.2498x; 1.2498x over previous
"""Trainium2 Bass kernel for the Mamba-style block nn_Block_29721173688983.

Strategy: tensor-parallel over d_inner (2048 channels / 8 cores = 256 each).
Per core: RMSNorm (redundant, rsqrt via Act table), its w_in column slice in
bf16, depthwise conv as diagonal bf16 matmuls, silu, partial x_proj
contraction -> two chunked bf16 AllReduces (pipelined with compute), delta
via Act Softplus, selective scan via DVE/Pool tensor_tensor_scan (f32
coefficients, bf16 inputs/outputs; L chunked in 2x1024 with chained state),
y = sum_n via bf16 identity-matmul PSUM accumulation consumed per-chunk,
out_proj bf16 partial matmul streamed per chunk.  Host sums the 8 bf16
partial outputs in f32 and adds the residual.

kernel(**inputs) takes the FULL unsharded inputs from setup_inputs() and
returns the FULL (1, 2048, 1024) output.
"""

import sys

sys.path.insert(0, "/opt/trn_rl_repo")

from contextlib import ExitStack

import numpy as np

import concourse.bacc as bacc
import concourse.bass as bass
import concourse.tile as tile
from concourse import mybir
from concourse.bass_utils import run_bass_kernel_spmd

F32 = mybir.dt.float32
F32R = mybir.dt.float32r
BF16 = mybir.dt.bfloat16
AF = mybir.ActivationFunctionType
OP = mybir.AluOpType

CORES = 8
D = 1024
DI = 2048
CL = DI // CORES          # 256 channels per core
NB = CL // 128            # 2 channel blocks
NST = 16                  # d_state
DTR = 64                  # dt_rank
KCONV = 4
EPS = 1e-5
NPROJ = DTR + 2 * NST     # 96


class _StopBuild(Exception):
    pass


def build(L=2048, collective=True, stop_after="D", reps=1, pool_mod=8):
    """Emit the SPMD single-core program (same program on all 8 cores).

    pool_mod: z-mult (n,cb) goes to Pool unless idx % pool_mod == pool_mod-1.
    """
    LTS = 512                  # psum free-dim tile
    NLT = L // LTS             # 4
    KB = D // 128              # 8 k-blocks for the w_in matmul
    LC = min(1024, L)          # phase-C scan chunk
    NCH = L // LC              # 2
    LPC = LC // LTS            # psum tiles per chunk (2)

    nc = bacc.Bacc("TRN2", target_bir_lowering=False, debug=False,
                   num_devices=CORES if collective else 1)

    def din(name, shape, dt=F32):
        return nc.dram_tensor(name, shape, dt, kind="ExternalInput").ap()

    xT_d = din("xT", [D, L], BF16)
    w_in_d = din("w_in_pack", [128, KB * 512], BF16)        # [p, (kb, m*128)]
    cbias_d = din("cbias_pack", [128, NB])
    A_d = din("A_pack", [128, NB * NST])
    D_d = din("D_pack", [128, NB])
    wxp_d = din("wxp_pack", [128, NB * NPROJ], BF16)
    wdt_d = din("wdt_loc", [DTR, CL], BF16)
    bdt_d = din("bdt_pack", [128, NB])
    bdtn_d = din("bdtn_pack", [128, NB])
    wout_d = din("wout_pack", [128, NB * D], BF16)
    ident_d = din("ident", [128, 128], BF16)
    ones_d = din("ones_in", [128, 1], F32R)
    cwdiag_d = din("cwdiag_pack", [128, NB * KCONV * 128], BF16)

    pout_d = nc.dram_tensor("part_out", [D, L], BF16, kind="ExternalOutput").ap()

    with tile.TileContext(nc) as tc:
      with ExitStack() as ctx:
        try:
            # ---- persistent pools ----
            cpool = ctx.enter_context(tc.tile_pool(name="consts", bufs=1))
            drpool = ctx.enter_context(
                tc.tile_pool(name="dram", bufs=1, space="DRAM"))

            cbias_t = cpool.tile([128, NB], F32)
            A_t = cpool.tile([128, NB * NST], F32)
            Aneg_t = cpool.tile([128, NB * NST], F32)
            D_t = cpool.tile([128, NB], F32)
            wxp_t = cpool.tile([128, NB * NPROJ], BF16)
            wdt_t = cpool.tile([DTR, CL], BF16)
            bdt_t = cpool.tile([128, NB], F32)
            bdtn_t = cpool.tile([128, NB], F32)
            wout_t = cpool.tile([128, NB * D], BF16)
            ident_t = cpool.tile([128, 128], BF16)
            ones_t = cpool.tile([128, 1], F32R)
            eps_t = cpool.tile([1, 1], F32)
            cwdiag_t = cpool.tile([128, NB * KCONV * 128], BF16)
            w_in_t = cpool.tile([128, KB * 512], BF16)

            nc.sync.dma_start(cbias_t[:], cbias_d[:])
            nc.sync.dma_start(A_t[:], A_d[:])
            nc.sync.dma_start(D_t[:], D_d[:])
            nc.sync.dma_start(wxp_t[:], wxp_d[:])
            nc.sync.dma_start(wdt_t[:], wdt_d[:])
            nc.sync.dma_start(bdt_t[:], bdt_d[:])
            nc.sync.dma_start(bdtn_t[:], bdtn_d[:])
            nc.sync.dma_start(wout_t[:], wout_d[:])
            nc.sync.dma_start(ident_t[:], ident_d[:])
            nc.sync.dma_start(ones_t[:], ones_d[:])
            nc.sync.dma_start(cwdiag_t[:], cwdiag_d[:])
            nc.sync.dma_start(w_in_t[:, 0:KB * 256], w_in_d[:, 0:KB * 256])
            nc.sync.dma_start(w_in_t[:, KB * 256:], w_in_d[:, KB * 256:])

            nc.vector.memset(eps_t[:], EPS)

            # A = -exp(A_log)
            nc.scalar.activation(Aneg_t[:], A_t[:], AF.Exp)
            nc.scalar.mul(Aneg_t[:], Aneg_t[:], -1.0)

            for rep in range(reps):
              with ExitStack() as rctx:
                # per-rep persistent activations (live through phase C)
                ppool = rctx.enter_context(
                    tc.tile_pool(name=f"persist{rep}", bufs=1))
                ar_in = [drpool.tile([NPROJ, LC], BF16, name=f"ar_in{ch}_{rep}")
                         for ch in range(NCH)]
                ar_out = [drpool.tile([NPROJ, LC], BF16, addr_space="Shared",
                                      name=f"ar_out{ch}_{rep}")
                          for ch in range(NCH)]
                rinv_dram = drpool.tile([1, L], F32, name=f"rinv_dram{rep}")
                res_silu = [ppool.tile([128, L], BF16,
                                       name=f"res_silu{i}_{rep}")
                            for i in range(NB)]
                rinv_bc = ppool.tile([128, L], F32, name=f"rinv_bc{rep}")

                # xs_pad lives only through A/B: right-side pool
                padpool_ctx = ExitStack()
                padpool = padpool_ctx.enter_context(
                    tc.tile_pool(name=f"pad{rep}", bufs=1, side="right"))
                xs_pad = [padpool.tile([128, L + KCONV - 1], BF16,
                                       name=f"xs_pad{i}_{rep}")
                          for i in range(NB)]
                for cb in range(NB):
                    nc.gpsimd.memset(xs_pad[cb][:, 0:KCONV - 1], 0.0)

                # ================= Phase A =================
                # Load bf16 xT; sumsq via square + ones-matmul (lt-outer to
                # hold only 2 ss psum banks); rinv via Act Rsqrt table; main
                # matmul on UNSCALED xT, rinv applied on PSUM evacuation.
                with ExitStack() as actx:
                    xtpool = actx.enter_context(tc.tile_pool(name="xt", bufs=KB))
                    sqpool = actx.enter_context(tc.tile_pool(name="sq", bufs=3))
                    sspool = actx.enter_context(
                        tc.tile_pool(name="ps_ss", bufs=2, space="PSUM"))
                    mmpool = actx.enter_context(
                        tc.tile_pool(name="ps_mm", bufs=4, space="PSUM"))
                    rowpool = actx.enter_context(tc.tile_pool(name="rows", bufs=1))
                    evpool = actx.enter_context(tc.tile_pool(name="ev", bufs=3))

                    xt_ts = []
                    for kb in range(KB):
                        xt = xtpool.tile([128, L], BF16, tag="xt")
                        nc.sync.dma_start(xt[:], xT_d[bass.ts(kb, 128), :])
                        xt_ts.append(xt)

                    rinv_row = rowpool.tile([1, L], F32)
                    for lt in range(NLT):
                        ss_ps = sspool.tile([1, LTS], F32, tag="ss")
                        for kb in range(KB):
                            sq = sqpool.tile([128, LTS], F32R, tag="sq")
                            nc.scalar.square(sq[:], xt_ts[kb][:, bass.ts(lt, LTS)])
                            nc.tensor.matmul(
                                ss_ps[:], ones_t[:], sq[:],
                                start=(kb == 0), stop=(kb == KB - 1))
                        # rinv = 1/sqrt(ss/D + EPS): Act Sqrt + DVE recip
                        nc.scalar.activation(
                            rinv_row[:, bass.ts(lt, LTS)], ss_ps[:],
                            AF.Sqrt, bias=eps_t[:], scale=1.0 / D)
                        nc.vector.reciprocal(
                            rinv_row[:, bass.ts(lt, LTS)],
                            rinv_row[:, bass.ts(lt, LTS)])
                        nc.sync.dma_start(
                            rinv_dram[:, bass.ts(lt, LTS)],
                            rinv_row[:, bass.ts(lt, LTS)])
                        nc.sync.dma_start(
                            rinv_bc[:, bass.ts(lt, LTS)],
                            rinv_dram[:, bass.ts(lt, LTS)]
                            .partition_broadcast(128))

                    # main matmul on raw xT; scale by rinv on evacuation.
                    for m in range(2 * NB):
                        for lt in range(NLT):
                            mm_ps = mmpool.tile([128, LTS], F32, tag="mm")
                            for kb in range(KB):
                                nc.tensor.matmul(
                                    mm_ps[:],
                                    w_in_t[:, kb * 512 + m * 128:
                                           kb * 512 + (m + 1) * 128],
                                    xt_ts[kb][:, bass.ts(lt, LTS)],
                                    start=(kb == 0), stop=(kb == KB - 1))
                            if m < NB:
                                nc.vector.tensor_mul(
                                    xs_pad[m][:, KCONV - 1 + lt * LTS:
                                              KCONV - 1 + (lt + 1) * LTS],
                                    mm_ps[:], rinv_bc[:, bass.ts(lt, LTS)])
                            else:
                                ev = evpool.tile([128, LTS], BF16, tag="ev")
                                nc.vector.tensor_mul(
                                    ev[:], mm_ps[:], rinv_bc[:, bass.ts(lt, LTS)])
                                nc.scalar.activation(
                                    res_silu[m - NB][:, bass.ts(lt, LTS)], ev[:],
                                    AF.Silu)

                if stop_after == "A":
                    padpool_ctx.close()
                    raise _StopBuild()

                # ================= Phase B =================
                apool = rctx.enter_context(tc.tile_pool(name=f"acts{rep}", bufs=1))
                xs_silu = [apool.tile([128, L], BF16, name=f"xs_silu{i}_{rep}")
                           for i in range(NB)]
                with ExitStack() as bctx:
                    prpool = bctx.enter_context(tc.tile_pool(name="proj", bufs=1))
                    pspool = bctx.enter_context(
                        tc.tile_pool(name="ps_b", bufs=2, space="PSUM"))

                    # depthwise causal conv on PE: 4 PSUM-accumulated
                    # diagonal matmuls per (cb, lt); Silu on evacuation.
                    for cb in range(NB):
                        for lt in range(NLT):
                            c_ps = pspool.tile([128, LTS], F32, tag="cps")
                            for j in range(KCONV):
                                nc.tensor.matmul(
                                    c_ps[:],
                                    cwdiag_t[:, (cb * KCONV + j) * 128:
                                             (cb * KCONV + j + 1) * 128],
                                    xs_pad[cb][:, j + lt * LTS:
                                               j + lt * LTS + LTS],
                                    start=(j == 0), stop=(j == KCONV - 1))
                            nc.scalar.activation(
                                xs_silu[cb][:, bass.ts(lt, LTS)], c_ps[:],
                                AF.Silu, bias=cbias_t[:, cb:cb + 1])

                    padpool_ctx.close()

                    # x_proj per chunk; chunked AllReduce so ch0's collective
                    # overlaps ch1's x_proj (and phase C ch0 work overlaps
                    # ch1's collective).
                    proj_sb = prpool.tile([NPROJ, L], BF16)
                    for ch in range(NCH):
                        for q in range(LPC):
                            lt = ch * LPC + q
                            pr_ps = pspool.tile([NPROJ, LTS], F32, tag="prps")
                            for cb in range(NB):
                                nc.tensor.matmul(
                                    pr_ps[:],
                                    wxp_t[:, cb * NPROJ:(cb + 1) * NPROJ],
                                    xs_silu[cb][:, bass.ts(lt, LTS)],
                                    start=(cb == 0), stop=(cb == NB - 1))
                            nc.scalar.copy(proj_sb[:, bass.ts(lt, LTS)], pr_ps[:])
                        nc.sync.dma_start(
                            ar_in[ch][:], proj_sb[:, bass.ts(ch, LC)])
                        if collective:
                            nc.gpsimd.collective_compute(
                                "AllReduce", OP.add,
                                replica_groups=[list(range(CORES))],
                                ins=[ar_in[ch].opt()], outs=[ar_out[ch].opt()])
                        else:
                            nc.sync.dma_start(ar_out[ch][:], ar_in[ch][:])

                if stop_after == "B":
                    raise _StopBuild()

                # ========== Phase C: delta, scan, y, fin, out_proj ==========
                # Chunked over NCH chunks of LC; scan state chained via a
                # [128, NB*NST] state tile.  Scans split DVE/Pool by pool_mod.
                state_t = rctx.enter_context(
                    tc.tile_pool(name=f"st{rep}", bufs=1)).tile(
                        [128, NB * NST], F32, name=f"state{rep}")

                with ExitStack() as cctx:
                    pdpool = cctx.enter_context(tc.tile_pool(name="pd", bufs=2))
                    dpool = cctx.enter_context(tc.tile_pool(name="dl", bufs=2))
                    upool = cctx.enter_context(tc.tile_pool(name="ul", bufs=2))
                    bcpool = cctx.enter_context(tc.tile_pool(name="bc", bufs=3))
                    dapool = cctx.enter_context(tc.tile_pool(name="da", bufs=3))
                    dxpool = cctx.enter_context(tc.tile_pool(name="dx", bufs=3))
                    yspool = cctx.enter_context(tc.tile_pool(name="ys", bufs=3))
                    zpool = cctx.enter_context(tc.tile_pool(name="zz", bufs=3))
                    fpool = cctx.enter_context(tc.tile_pool(name="fi", bufs=3))
                    popool = cctx.enter_context(tc.tile_pool(name="po", bufs=3))
                    dps_pool = cctx.enter_context(
                        tc.tile_pool(name="ps_d", bufs=2, space="PSUM"))
                    ypspool = cctx.enter_context(
                        tc.tile_pool(name="ps_y", bufs=1, space="PSUM"))
                    opspool = cctx.enter_context(
                        tc.tile_pool(name="ps_o", bufs=2, space="PSUM"))

                    for ch in range(NCH):
                        cs = slice(ch * LC, (ch + 1) * LC)
                        # delta & u per channel block for this chunk
                        pd_sb = pdpool.tile([DTR, LC], BF16, tag="pd")
                        nc.sync.dma_start(pd_sb[:], ar_out[ch][0:DTR, :])
                        delta = []
                        u_t = []
                        for cb in range(NB):
                            dl = dpool.tile([128, LC], F32, tag=f"dl{cb}")
                            for q in range(LPC):
                                qs = bass.ts(q, LTS)
                                d_ps = dps_pool.tile([128, LTS], F32, tag="dps")
                                nc.tensor.matmul(
                                    d_ps[:], wdt_t[:, bass.ts(cb, 128)],
                                    pd_sb[:, qs], start=True, stop=True)
                                # softplus(z) = z + ln(1 + exp(-z)); Exp and
                                # Ln share act-table set 6 with phase-C Exp.
                                e_t = dpool.tile([128, LTS], F32, tag="e")
                                nc.scalar.activation(
                                    e_t[:], d_ps[:], AF.Exp, scale=-1.0,
                                    bias=bdtn_t[:, cb:cb + 1])
                                t_t = dpool.tile([128, LTS], F32, tag="t")
                                nc.scalar.activation(
                                    t_t[:], e_t[:], AF.Ln, bias=1.0)
                                nc.vector.scalar_tensor_tensor(
                                    dl[:, qs], d_ps[:], bdt_t[:, cb:cb + 1],
                                    t_t[:], op0=OP.add, op1=OP.add)
                            delta.append(dl)
                            ul = upool.tile([128, LC], BF16, tag=f"ul{cb}")
                            nc.vector.tensor_mul(
                                ul[:], dl[:], xs_silu[cb][:, cs])
                            u_t.append(ul)

                        y_ps = [[ypspool.tile([128, LTS], F32, tag=f"yps{cb}_{q}",
                                              name=f"yps{cb}_{q}_{ch}_{rep}")
                                 for q in range(LPC)] for cb in range(NB)]

                        for n in range(NST):
                            Bb = bcpool.tile([128, LC], BF16, tag="Bb")
                            Cb = bcpool.tile([128, LC], BF16, tag="Cb")
                            nc.sync.dma_start(
                                Bb[:], ar_out[ch][DTR + n:DTR + n + 1, :]
                                .partition_broadcast(128))
                            nc.sync.dma_start(
                                Cb[:], ar_out[ch][DTR + NST + n:DTR + NST + n + 1, :]
                                .partition_broadcast(128))
                            for cb in range(NB):
                                idx = n * NB + cb
                                da = dapool.tile([128, LC], F32, tag="da")
                                nc.scalar.activation(
                                    da[:], delta[cb][:], AF.Exp,
                                    scale=Aneg_t[:, cb * NST + n:cb * NST + n + 1])
                                dbx = dxpool.tile([128, LC], BF16, tag="dbx")
                                nc.vector.tensor_mul(dbx[:], u_t[cb][:], Bb[:])
                                ys = yspool.tile([128, LC], BF16, tag="ys")
                                nc.vector.tensor_tensor_scan(
                                    ys[:], da[:], dbx[:],
                                    0.0 if ch == 0 else state_t[:, idx:idx + 1],
                                    op0=OP.mult, op1=OP.add)
                                if ch < NCH - 1:
                                    nc.vector.tensor_copy(
                                        state_t[:, idx:idx + 1],
                                        ys[:, LC - 1:LC])
                                z = zpool.tile([128, LC], BF16, tag="z")
                                zeng = (nc.gpsimd if idx % pool_mod != pool_mod - 1
                                        else nc.vector)
                                zeng.tensor_tensor(z[:], ys[:], Cb[:], op=OP.mult)
                                for q in range(LPC):
                                    nc.tensor.matmul(
                                        y_ps[cb][q][:], ident_t[:],
                                        z[:, bass.ts(q, LTS)],
                                        start=(n == 0), stop=(n == NST - 1))

                        # fin + out_proj for this chunk
                        fin = []
                        for cb in range(NB):
                            fl = fpool.tile([128, LC], BF16, tag=f"fin{cb}")
                            for q in range(LPC):
                                lt = ch * LPC + q
                                tmp = fpool.tile([128, LTS], BF16, tag="ftmp")
                                nc.vector.scalar_tensor_tensor(
                                    tmp[:],
                                    xs_silu[cb][:, bass.ts(lt, LTS)],
                                    D_t[:, cb:cb + 1], y_ps[cb][q][:],
                                    op0=OP.mult, op1=OP.add)
                                nc.vector.tensor_mul(
                                    fl[:, bass.ts(q, LTS)], tmp[:],
                                    res_silu[cb][:, bass.ts(lt, LTS)])
                            fin.append(fl)

                        if stop_after == "C" and ch == NCH - 1:
                            raise _StopBuild()

                        for m in range(D // 128):
                            for q in range(LPC):
                                lt = ch * LPC + q
                                o_ps = opspool.tile([128, LTS], F32, tag="ops")
                                for cb in range(NB):
                                    nc.tensor.matmul(
                                        o_ps[:],
                                        wout_t[:, cb * D + m * 128:
                                               cb * D + (m + 1) * 128],
                                        fin[cb][:, bass.ts(q, LTS)],
                                        start=(cb == 0), stop=(cb == NB - 1))
                                po = popool.tile([128, LTS], BF16, tag="po")
                                nc.scalar.copy(po[:], o_ps[:])
                                nc.sync.dma_start(
                                    pout_d[bass.ts(m, 128), bass.ts(lt, LTS)],
                                    po[:])
        except _StopBuild:
            pass

    nc.compile()
    return nc


def _bf16(a):
    return np.asarray(a, dtype=mybir.dt.np(BF16))


def host_prep(inputs, L=2048):
    """Slice/replicate the full inputs into 8 per-core input maps."""
    x = np.asarray(inputs["x"], np.float32)
    norm_scale = np.asarray(inputs["norm_scale"], np.float32)
    w_in = np.asarray(inputs["w_in"], np.float32)
    conv_w = np.asarray(inputs["conv_w"], np.float32)
    conv_b = np.asarray(inputs["conv_b"], np.float32)
    A_log = np.asarray(inputs["A_log"], np.float32)
    D_in = np.asarray(inputs["D"], np.float32)
    w_xproj = np.asarray(inputs["w_xproj"], np.float32)
    w_dt = np.asarray(inputs["w_dt"], np.float32)
    b_dt = np.asarray(inputs["b_dt"], np.float32)
    w_out = np.asarray(inputs["w_out"], np.float32)

    x2 = x[0, :L, :]                              # (L, D)
    xT = np.ascontiguousarray(x2.T)               # (D, L)
    w_in_s = w_in * norm_scale[:, None]
    ident = np.eye(128, dtype=np.float32)
    KB = D // 128

    def pack_nb(v):                                # (CL,) -> [128, NB]
        return np.ascontiguousarray(v.reshape(NB, 128).T)

    in_maps = []
    for k in range(CORES):
        sl = slice(k * CL, (k + 1) * CL)
        wi = np.concatenate(
            [w_in_s[:, k * CL:(k + 1) * CL],
             w_in_s[:, DI + k * CL:DI + (k + 1) * CL]], axis=1)  # (D, 512)
        w_in_pack = np.ascontiguousarray(
            wi.reshape(KB, 128, 512).transpose(1, 0, 2).reshape(128, KB * 512))
        cw = conv_w[:, 0, sl]                     # (4, CL)
        A_pack = np.ascontiguousarray(
            A_log[sl].reshape(NB, 128, NST).transpose(1, 0, 2)
            .reshape(128, NB * NST))
        wxp_pack = np.ascontiguousarray(
            w_xproj[sl].reshape(NB, 128, DTR + 2 * NST)
            .transpose(1, 0, 2).reshape(128, NB * (DTR + 2 * NST)))
        wout_pack = np.ascontiguousarray(
            w_out[sl].reshape(NB, 128, D).transpose(1, 0, 2)
            .reshape(128, NB * D))
        in_maps.append({
            "xT": _bf16(xT),
            "w_in_pack": _bf16(w_in_pack),
            "cbias_pack": pack_nb(conv_b[sl]),
            "A_pack": A_pack,
            "D_pack": pack_nb(D_in[sl]),
            "wxp_pack": _bf16(wxp_pack),
            "wdt_loc": _bf16(np.ascontiguousarray(w_dt[:, sl])),
            "bdt_pack": pack_nb(b_dt[sl]),
            "bdtn_pack": pack_nb(-b_dt[sl]),
            "wout_pack": _bf16(wout_pack),
            "ident": _bf16(ident),
            "ones_in": np.ones((128, 1), np.float32),
            "cwdiag_pack": _bf16(np.concatenate(
                [np.diag(cw[j, cb * 128:(cb + 1) * 128]).astype(np.float32)
                 for cb in range(NB) for j in range(KCONV)], axis=1)),
        })
    return in_maps


def combine(inputs, results, L=2048):
    """Host unshard: sum the 8 partial outputs, add residual."""
    x = np.asarray(inputs["x"], np.float32)
    acc = np.zeros((D, L), np.float32)
    for r in results:
        acc += np.asarray(r["part_out"], np.float32)
    out = x[0, :L, :] + acc.T
    return out[None].astype(np.float32)


_CACHE = {}


def kernel(**inputs):
    if "nc" not in _CACHE:
        _CACHE["nc"] = build()
    nc = _CACHE["nc"]
    in_maps = host_prep(inputs)
    res = run_bass_kernel_spmd(nc, in_maps, list(range(CORES)))
    return combine(inputs, res.results)


if __name__ == "__main__":
    import reference

    inputs = reference.setup_inputs()
    inputs = {k: np.asarray(v) for k, v in inputs.items()}
    expected = np.asarray(reference.reference(**inputs))
    actual = kernel(**inputs)
    err = np.abs(actual - expected).max() / np.abs(expected).max()
    print("Relative error:", err)


# revision 28
# speedup vs baseline: 1.8917x; 1.5136x over previous
"""Trainium2 Bass kernel for the Mamba-style block nn_Block_29721173688983.

Strategy: tensor-parallel over d_inner (2048 channels / 8 cores = 256 each).
Per core: RMSNorm (redundant, rsqrt via Act table), its w_in column slice in
bf16, depthwise conv as diagonal bf16 matmuls, silu, partial x_proj
contraction -> two chunked bf16 AllReduces (pipelined with compute), delta
via Act Softplus, selective scan via DVE/Pool tensor_tensor_scan (f32
coefficients, bf16 inputs/outputs; L chunked in 2x1024 with chained state),
y = sum_n via bf16 identity-matmul PSUM accumulation consumed per-chunk,
out_proj bf16 partial matmul streamed per chunk.  Host sums the 8 bf16
partial outputs in f32 and adds the residual.

kernel(**inputs) takes the FULL unsharded inputs from setup_inputs() and
returns the FULL (1, 2048, 1024) output.
"""

import sys

sys.path.insert(0, "/opt/trn_rl_repo")

from contextlib import ExitStack

import numpy as np

import concourse.bacc as bacc
import concourse.bass as bass
import concourse.tile as tile
from concourse import mybir
from concourse.bass_utils import run_bass_kernel_spmd

F32 = mybir.dt.float32
F32R = mybir.dt.float32r
BF16 = mybir.dt.bfloat16
AF = mybir.ActivationFunctionType
OP = mybir.AluOpType

CORES = 8
D = 1024
DI = 2048
CL = DI // CORES          # 256 channels per core
NB = CL // 128            # 2 channel blocks
NST = 16                  # d_state
DTR = 64                  # dt_rank
KCONV = 4
EPS = 1e-5
NPROJ = DTR + 2 * NST     # 96


class _StopBuild(Exception):
    pass


def build(L=2048, collective=True, stop_after="D", reps=1, pool_mod=8):
    """Emit the SPMD single-core program (same program on all 8 cores).

    pool_mod: z-mult (n,cb) goes to Pool unless idx % pool_mod == pool_mod-1.
    """
    LTS = 512                  # psum free-dim tile
    NLT = L // LTS             # 4
    KB = D // 128              # 8 k-blocks for the w_in matmul
    LC = min(512, L)           # chunk size: collective + scan granularity
    NCH = L // LC              # 4
    LPC = LC // LTS            # psum tiles per chunk (1)

    nc = bacc.Bacc("TRN2", target_bir_lowering=False, debug=False,
                   num_devices=CORES if collective else 1)

    def din(name, shape, dt=F32):
        return nc.dram_tensor(name, shape, dt, kind="ExternalInput").ap()

    xT_d = din("xT", [D, L], BF16)
    w_in_d = din("w_in_pack", [128, KB * 512], BF16)        # [p, (kb, m*128)]
    cbias_d = din("cbias_pack", [128, NB])
    A_d = din("A_pack", [128, NB * NST])
    D_d = din("D_pack", [128, NB])
    wxp_d = din("wxp_pack", [128, NB * NPROJ], BF16)
    wdt_d = din("wdt_loc", [DTR, CL], BF16)
    bdt_d = din("bdt_pack", [128, NB])
    bdtn_d = din("bdtn_pack", [128, NB])
    wout_d = din("wout_pack", [128, NB * D], BF16)
    ident_d = din("ident", [128, 128], BF16)
    ones_d = din("ones_in", [128, 1], F32R)
    cwdiag_d = din("cwdiag_pack", [128, NB * KCONV * 128], BF16)

    pout_d = nc.dram_tensor("part_out", [D, L], BF16, kind="ExternalOutput").ap()

    with tile.TileContext(nc) as tc:
      with ExitStack() as ctx:
        try:
            # ---- persistent pools ----
            cpool = ctx.enter_context(tc.tile_pool(name="consts", bufs=1))
            drpool = ctx.enter_context(
                tc.tile_pool(name="dram", bufs=1, space="DRAM"))

            cbias_t = cpool.tile([128, NB], F32)
            A_t = cpool.tile([128, NB * NST], F32)
            Aneg_t = cpool.tile([128, NB * NST], F32)
            D_t = cpool.tile([128, NB], F32)
            wxp_t = cpool.tile([128, NB * NPROJ], BF16)
            wdt_t = cpool.tile([DTR, CL], BF16)
            bdt_t = cpool.tile([128, NB], F32)
            bdtn_t = cpool.tile([128, NB], F32)
            wout_t = cpool.tile([128, NB * D], BF16)
            ident_t = cpool.tile([128, 128], BF16)
            ones_t = cpool.tile([128, 1], F32R)
            eps_t = cpool.tile([1, 1], F32)
            cwdiag_t = cpool.tile([128, NB * KCONV * 128], BF16)
            w_in_t = cpool.tile([128, KB * 512], BF16)

            nc.sync.dma_start(cbias_t[:], cbias_d[:])
            nc.sync.dma_start(A_t[:], A_d[:])
            nc.sync.dma_start(D_t[:], D_d[:])
            nc.sync.dma_start(wxp_t[:], wxp_d[:])
            nc.sync.dma_start(wdt_t[:], wdt_d[:])
            nc.sync.dma_start(bdt_t[:], bdt_d[:])
            nc.sync.dma_start(bdtn_t[:], bdtn_d[:])
            nc.sync.dma_start(wout_t[:], wout_d[:])
            nc.sync.dma_start(ident_t[:], ident_d[:])
            nc.sync.dma_start(ones_t[:], ones_d[:])
            nc.sync.dma_start(cwdiag_t[:], cwdiag_d[:])
            nc.sync.dma_start(w_in_t[:, 0:KB * 256], w_in_d[:, 0:KB * 256])
            nc.sync.dma_start(w_in_t[:, KB * 256:], w_in_d[:, KB * 256:])

            nc.vector.memset(eps_t[:], EPS)

            # A = -exp(A_log)
            nc.scalar.activation(Aneg_t[:], A_t[:], AF.Exp)
            nc.scalar.mul(Aneg_t[:], Aneg_t[:], -1.0)

            for rep in range(reps):
              with ExitStack() as rctx:
                # per-rep persistent activations (live through phase C)
                ppool = rctx.enter_context(
                    tc.tile_pool(name=f"persist{rep}", bufs=1))
                ar_in = [drpool.tile([NPROJ, LC], BF16, name=f"ar_in{ch}_{rep}")
                         for ch in range(NCH)]
                ar_out = [drpool.tile([NPROJ, LC], BF16, addr_space="Shared",
                                      name=f"ar_out{ch}_{rep}")
                          for ch in range(NCH)]
                rinv_dram = drpool.tile([1, L], F32, name=f"rinv_dram{rep}")
                res_silu = [ppool.tile([128, L], BF16,
                                       name=f"res_silu{i}_{rep}")
                            for i in range(NB)]
                rinv_bc = ppool.tile([128, L], F32, name=f"rinv_bc{rep}")

                # xs_pad lives only through A/B: right-side pool
                padpool_ctx = ExitStack()
                padpool = padpool_ctx.enter_context(
                    tc.tile_pool(name=f"pad{rep}", bufs=1, side="right"))
                xs_pad = [padpool.tile([128, L + KCONV - 1], BF16,
                                       name=f"xs_pad{i}_{rep}")
                          for i in range(NB)]
                for cb in range(NB):
                    nc.gpsimd.memset(xs_pad[cb][:, 0:KCONV - 1], 0.0)

                apool = rctx.enter_context(
                    tc.tile_pool(name=f"acts{rep}", bufs=1))
                xs_silu = [apool.tile([128, L], BF16, name=f"xs_silu{i}_{rep}")
                           for i in range(NB)]
                proj_sb = apool.tile([NPROJ, L], BF16, name=f"proj{rep}")

                # ================= Phase A =================
                # Load bf16 xT; sumsq via square + ones-matmul (lt-outer to
                # hold only 2 ss psum banks); rinv via Act Rsqrt table; main
                # matmul on UNSCALED xT, rinv applied on PSUM evacuation.
                with ExitStack() as actx:
                    xtpool = actx.enter_context(tc.tile_pool(name="xt", bufs=KB))
                    sqpool = actx.enter_context(tc.tile_pool(name="sq", bufs=3))
                    sspool = actx.enter_context(
                        tc.tile_pool(name="ps_ss", bufs=2, space="PSUM"))
                    mmpool = actx.enter_context(
                        tc.tile_pool(name="ps_mm", bufs=2, space="PSUM"))
                    rowpool = actx.enter_context(tc.tile_pool(name="rows", bufs=1))
                    evpool = actx.enter_context(tc.tile_pool(name="ev", bufs=3))

                    xt_ts = []
                    for kb in range(KB):
                        xt = xtpool.tile([128, L], BF16, tag="xt")
                        xt_ts.append(xt)
                    half = L // 2
                    for h in range(2):
                        for kb in range(KB):
                            nc.sync.dma_start(
                                xt_ts[kb][:, h * half:(h + 1) * half],
                                xT_d[bass.ts(kb, 128), h * half:(h + 1) * half])

                    rinv_row = rowpool.tile([1, L], F32)
                    for lt in range(NLT):
                        ss_ps = sspool.tile([1, LTS], F32, tag="ss")
                        for kb in range(KB):
                            sq = sqpool.tile([128, LTS], F32R, tag="sq")
                            nc.scalar.square(sq[:], xt_ts[kb][:, bass.ts(lt, LTS)])
                            nc.tensor.matmul(
                                ss_ps[:], ones_t[:], sq[:],
                                start=(kb == 0), stop=(kb == KB - 1))
                        # rinv = 1/sqrt(ss/D + EPS): Act Sqrt + DVE recip
                        nc.scalar.activation(
                            rinv_row[:, bass.ts(lt, LTS)], ss_ps[:],
                            AF.Sqrt, bias=eps_t[:], scale=1.0 / D)
                        nc.vector.reciprocal(
                            rinv_row[:, bass.ts(lt, LTS)],
                            rinv_row[:, bass.ts(lt, LTS)])
                        nc.sync.dma_start(
                            rinv_dram[:, bass.ts(lt, LTS)],
                            rinv_row[:, bass.ts(lt, LTS)])
                        nc.sync.dma_start(
                            rinv_bc[:, bass.ts(lt, LTS)],
                            rinv_dram[:, bass.ts(lt, LTS)]
                            .partition_broadcast(128))

                    # Fused per-lt pipeline: w_in matmul -> conv -> x_proj ->
                    # chunked AllReduce, so the first collective fires while
                    # later lt chunks are still in the matmul.
                    for lt in range(NLT):
                        ls = bass.ts(lt, LTS)
                        for m in range(2 * NB):
                            mm_ps = mmpool.tile([128, LTS], F32, tag="mm")
                            for kb in range(KB):
                                nc.tensor.matmul(
                                    mm_ps[:],
                                    w_in_t[:, kb * 512 + m * 128:
                                           kb * 512 + (m + 1) * 128],
                                    xt_ts[kb][:, ls],
                                    start=(kb == 0), stop=(kb == KB - 1))
                            if m < NB:
                                nc.vector.tensor_mul(
                                    xs_pad[m][:, KCONV - 1 + lt * LTS:
                                              KCONV - 1 + (lt + 1) * LTS],
                                    mm_ps[:], rinv_bc[:, ls])
                            else:
                                ev = evpool.tile([128, LTS], BF16, tag="ev")
                                nc.vector.tensor_mul(
                                    ev[:], mm_ps[:], rinv_bc[:, ls])
                                nc.scalar.activation(
                                    res_silu[m - NB][:, ls], ev[:], AF.Silu)
                        # conv + silu for this lt
                        for cb in range(NB):
                            c_ps = mmpool.tile([128, LTS], F32, tag="cps")
                            for j in range(KCONV):
                                nc.tensor.matmul(
                                    c_ps[:],
                                    cwdiag_t[:, (cb * KCONV + j) * 128:
                                             (cb * KCONV + j + 1) * 128],
                                    xs_pad[cb][:, j + lt * LTS:
                                               j + lt * LTS + LTS],
                                    start=(j == 0), stop=(j == KCONV - 1))
                            nc.scalar.activation(
                                xs_silu[cb][:, ls], c_ps[:],
                                AF.Silu, bias=cbias_t[:, cb:cb + 1])
                        # x_proj + AllReduce for this lt chunk
                        pr_ps = mmpool.tile([NPROJ, LTS], F32, tag="prps")
                        for cb in range(NB):
                            nc.tensor.matmul(
                                pr_ps[:],
                                wxp_t[:, cb * NPROJ:(cb + 1) * NPROJ],
                                xs_silu[cb][:, ls],
                                start=(cb == 0), stop=(cb == NB - 1))
                        nc.scalar.copy(proj_sb[:, ls], pr_ps[:])
                        nc.sync.dma_start(ar_in[lt][:], proj_sb[:, ls])
                        if collective:
                            nc.gpsimd.collective_compute(
                                "AllReduce", OP.add,
                                replica_groups=[list(range(CORES))],
                                ins=[ar_in[lt].opt()], outs=[ar_out[lt].opt()])
                        else:
                            nc.sync.dma_start(ar_out[lt][:], ar_in[lt][:])

                    padpool_ctx.close()

                if stop_after in ("A", "B"):
                    raise _StopBuild()

                # ========== Phase C: delta, scan, y, fin, out_proj ==========
                # Chunked over NCH chunks of LC; scan state chained via a
                # [128, NB*NST] state tile.  Scans split DVE/Pool by pool_mod.
                state_t = rctx.enter_context(
                    tc.tile_pool(name=f"st{rep}", bufs=1)).tile(
                        [128, NB * NST], F32, name=f"state{rep}")

                with ExitStack() as cctx:
                    pdpool = cctx.enter_context(tc.tile_pool(name="pd", bufs=2))
                    dpool = cctx.enter_context(tc.tile_pool(name="dl", bufs=2))
                    upool = cctx.enter_context(tc.tile_pool(name="ul", bufs=2))
                    bcpool = cctx.enter_context(tc.tile_pool(name="bc", bufs=3))
                    dapool = cctx.enter_context(tc.tile_pool(name="da", bufs=3))
                    dxpool = cctx.enter_context(tc.tile_pool(name="dx", bufs=3))
                    yspool = cctx.enter_context(tc.tile_pool(name="ys", bufs=3))
                    zpool = cctx.enter_context(tc.tile_pool(name="zz", bufs=3))
                    fpool = cctx.enter_context(tc.tile_pool(name="fi", bufs=3))
                    dps_pool = cctx.enter_context(
                        tc.tile_pool(name="ps_d", bufs=2, space="PSUM"))
                    ypspool = cctx.enter_context(
                        tc.tile_pool(name="ps_y", bufs=1, space="PSUM"))
                    opspool = cctx.enter_context(
                        tc.tile_pool(name="ps_o", bufs=2, space="PSUM"))

                    for ch in range(NCH):
                        cs = slice(ch * LC, (ch + 1) * LC)
                        # delta & u per channel block for this chunk
                        pd_sb = pdpool.tile([DTR, LC], BF16, tag="pd")
                        nc.sync.dma_start(pd_sb[:], ar_out[ch][0:DTR, :])
                        delta = []
                        u_t = []
                        for cb in range(NB):
                            dl = dpool.tile([128, LC], F32, tag=f"dl{cb}")
                            for q in range(LPC):
                                qs = bass.ts(q, LTS)
                                d_ps = dps_pool.tile([128, LTS], F32, tag="dps")
                                nc.tensor.matmul(
                                    d_ps[:], wdt_t[:, bass.ts(cb, 128)],
                                    pd_sb[:, qs], start=True, stop=True)
                                # softplus(z) = z + ln(1 + exp(-z)); Exp and
                                # Ln share act-table set 6 with phase-C Exp.
                                e_t = dpool.tile([128, LTS], F32, tag="e")
                                nc.scalar.activation(
                                    e_t[:], d_ps[:], AF.Exp, scale=-1.0,
                                    bias=bdtn_t[:, cb:cb + 1])
                                t_t = dpool.tile([128, LTS], F32, tag="t")
                                nc.scalar.activation(
                                    t_t[:], e_t[:], AF.Ln, bias=1.0)
                                nc.vector.scalar_tensor_tensor(
                                    dl[:, qs], d_ps[:], bdt_t[:, cb:cb + 1],
                                    t_t[:], op0=OP.add, op1=OP.add)
                            delta.append(dl)
                            ul = upool.tile([128, LC], BF16, tag=f"ul{cb}")
                            nc.vector.tensor_mul(
                                ul[:], dl[:], xs_silu[cb][:, cs])
                            u_t.append(ul)

                        y_ps = [[ypspool.tile([128, LTS], F32, tag=f"yps{cb}_{q}",
                                              name=f"yps{cb}_{q}_{ch}_{rep}")
                                 for q in range(LPC)] for cb in range(NB)]

                        for n in range(NST):
                            Bb = bcpool.tile([128, LC], BF16, tag="Bb")
                            Cb = bcpool.tile([128, LC], BF16, tag="Cb")
                            nc.sync.dma_start(
                                Bb[:], ar_out[ch][DTR + n:DTR + n + 1, :]
                                .partition_broadcast(128))
                            nc.sync.dma_start(
                                Cb[:], ar_out[ch][DTR + NST + n:DTR + NST + n + 1, :]
                                .partition_broadcast(128))
                            for cb in range(NB):
                                idx = n * NB + cb
                                da = dapool.tile([128, LC], F32, tag="da")
                                nc.scalar.activation(
                                    da[:], delta[cb][:], AF.Exp,
                                    scale=Aneg_t[:, cb * NST + n:cb * NST + n + 1])
                                dbx = dxpool.tile([128, LC], BF16, tag="dbx")
                                nc.vector.tensor_mul(dbx[:], u_t[cb][:], Bb[:])
                                ys = yspool.tile([128, LC], BF16, tag="ys")
                                nc.vector.tensor_tensor_scan(
                                    ys[:], da[:], dbx[:],
                                    0.0 if ch == 0 else state_t[:, idx:idx + 1],
                                    op0=OP.mult, op1=OP.add)
                                if ch < NCH - 1:
                                    nc.vector.tensor_copy(
                                        state_t[:, idx:idx + 1],
                                        ys[:, LC - 1:LC])
                                z = zpool.tile([128, LC], BF16, tag="z")
                                zeng = (nc.gpsimd if idx % pool_mod != pool_mod - 1
                                        else nc.vector)
                                zeng.tensor_tensor(z[:], ys[:], Cb[:], op=OP.mult)
                                for q in range(LPC):
                                    nc.tensor.matmul(
                                        y_ps[cb][q][:], ident_t[:],
                                        z[:, bass.ts(q, LTS)],
                                        start=(n == 0), stop=(n == NST - 1))

                        # fin + out_proj for this chunk
                        fin = []
                        for cb in range(NB):
                            fl = fpool.tile([128, LC], BF16, tag=f"fin{cb}")
                            for q in range(LPC):
                                lt = ch * LPC + q
                                tmp = fpool.tile([128, LTS], BF16, tag="ftmp")
                                nc.vector.scalar_tensor_tensor(
                                    tmp[:],
                                    xs_silu[cb][:, bass.ts(lt, LTS)],
                                    D_t[:, cb:cb + 1], y_ps[cb][q][:],
                                    op0=OP.mult, op1=OP.add)
                                nc.vector.tensor_mul(
                                    fl[:, bass.ts(q, LTS)], tmp[:],
                                    res_silu[cb][:, bass.ts(lt, LTS)])
                            fin.append(fl)

                        if stop_after == "C" and ch == NCH - 1:
                            raise _StopBuild()

                        for m in range(D // 128):
                            for q in range(LPC):
                                lt = ch * LPC + q
                                o_ps = opspool.tile([128, LTS], F32, tag="ops")
                                for cb in range(NB):
                                    nc.tensor.matmul(
                                        o_ps[:],
                                        wout_t[:, cb * D + m * 128:
                                               cb * D + (m + 1) * 128],
                                        fin[cb][:, bass.ts(q, LTS)],
                                        start=(cb == 0), stop=(cb == NB - 1))
                                po = fpool.tile([128, LTS], BF16, tag="po")
                                nc.scalar.copy(po[:], o_ps[:])
                                nc.sync.dma_start(
                                    pout_d[bass.ts(m, 128), bass.ts(lt, LTS)],
                                    po[:])
        except _StopBuild:
            pass

    nc.compile()
    return nc


def _bf16(a):
    return np.asarray(a, dtype=mybir.dt.np(BF16))


def host_prep(inputs, L=2048):
    """Slice/replicate the full inputs into 8 per-core input maps."""
    x = np.asarray(inputs["x"], np.float32)
    norm_scale = np.asarray(inputs["norm_scale"], np.float32)
    w_in = np.asarray(inputs["w_in"], np.float32)
    conv_w = np.asarray(inputs["conv_w"], np.float32)
    conv_b = np.asarray(inputs["conv_b"], np.float32)
    A_log = np.asarray(inputs["A_log"], np.float32)
    D_in = np.asarray(inputs["D"], np.float32)
    w_xproj = np.asarray(inputs["w_xproj"], np.float32)
    w_dt = np.asarray(inputs["w_dt"], np.float32)
    b_dt = np.asarray(inputs["b_dt"], np.float32)
    w_out = np.asarray(inputs["w_out"], np.float32)

    x2 = x[0, :L, :]                              # (L, D)
    xT = np.ascontiguousarray(x2.T)               # (D, L)
    w_in_s = w_in * norm_scale[:, None]
    ident = np.eye(128, dtype=np.float32)
    KB = D // 128

    def pack_nb(v):                                # (CL,) -> [128, NB]
        return np.ascontiguousarray(v.reshape(NB, 128).T)

    in_maps = []
    for k in range(CORES):
        sl = slice(k * CL, (k + 1) * CL)
        wi = np.concatenate(
            [w_in_s[:, k * CL:(k + 1) * CL],
             w_in_s[:, DI + k * CL:DI + (k + 1) * CL]], axis=1)  # (D, 512)
        w_in_pack = np.ascontiguousarray(
            wi.reshape(KB, 128, 512).transpose(1, 0, 2).reshape(128, KB * 512))
        cw = conv_w[:, 0, sl]                     # (4, CL)
        A_pack = np.ascontiguousarray(
            A_log[sl].reshape(NB, 128, NST).transpose(1, 0, 2)
            .reshape(128, NB * NST))
        wxp_pack = np.ascontiguousarray(
            w_xproj[sl].reshape(NB, 128, DTR + 2 * NST)
            .transpose(1, 0, 2).reshape(128, NB * (DTR + 2 * NST)))
        wout_pack = np.ascontiguousarray(
            w_out[sl].reshape(NB, 128, D).transpose(1, 0, 2)
            .reshape(128, NB * D))
        in_maps.append({
            "xT": _bf16(xT),
            "w_in_pack": _bf16(w_in_pack),
            "cbias_pack": pack_nb(conv_b[sl]),
            "A_pack": A_pack,
            "D_pack": pack_nb(D_in[sl]),
            "wxp_pack": _bf16(wxp_pack),
            "wdt_loc": _bf16(np.ascontiguousarray(w_dt[:, sl])),
            "bdt_pack": pack_nb(b_dt[sl]),
            "bdtn_pack": pack_nb(-b_dt[sl]),
            "wout_pack": _bf16(wout_pack),
            "ident": _bf16(ident),
            "ones_in": np.ones((128, 1), np.float32),
            "cwdiag_pack": _bf16(np.concatenate(
                [np.diag(cw[j, cb * 128:(cb + 1) * 128]).astype(np.float32)
                 for cb in range(NB) for j in range(KCONV)], axis=1)),
        })
    return in_maps


def combine(inputs, results, L=2048):
    """Host unshard: sum the 8 partial outputs, add residual."""
    x = np.asarray(inputs["x"], np.float32)
    acc = np.zeros((D, L), np.float32)
    for r in results:
        acc += np.asarray(r["part_out"], np.float32)
    out = x[0, :L, :] + acc.T
    return out[None].astype(np.float32)


_CACHE = {}


def kernel(**inputs):
    if "nc" not in _CACHE:
        _CACHE["nc"] = build()
    nc = _CACHE["nc"]
    in_maps = host_prep(inputs)
    res = run_bass_kernel_spmd(nc, in_maps, list(range(CORES)))
    return combine(inputs, res.results)


if __name__ == "__main__":
    import reference

    inputs = reference.setup_inputs()
    inputs = {k: np.asarray(v) for k, v in inputs.items()}
    expected = np.asarray(reference.reference(**inputs))
    actual = kernel(**inputs)
    err = np.abs(actual - expected).max() / np.abs(expected).max()
    print("Relative error:", err)
